# revision 12
# baseline (speedup 1.0000x reference)
"""Trainium2 Bass kernel for nn_ClusterlingLayer (ragged_sequence).

Computes, for B=131072 fibers against K=64 clusters:
  x_dis[b,k] = ||x_b||^2 + ||w_k||^2 - 2 x_b.w_k
  dice[b,k]  = 1 - (2*inter + s)/(nF + nC + s)   (inter = ragged ROI histogram dot)
  q = rownorm( 1 / (1 + x_dis*dice) )
Returns (q, x_dis) like the reference.

Sharding: data-parallel over B across 8 NeuronCores (16384 fibers/core).

Device strategy per 128-fiber subtile (fibers globally sorted by length and
dealt round-robin so all 8 cores share one compile-time profile):
 - per-fiber ROI histograms are built TRANSPOSED ([vocab, fiber]) in one
   GPSIMD local_scatter op per subtile: the host pre-groups each subtile's
   (fiber, bin, count) triples by bin; partition v scatters count into
   column fiber. This replaces the DVE compare-chains, the PE transpose
   and the PSUM->SBUF copy of the old design.
 - PE contracts histT with tbl2 = [1 - 2*histC^T | ones] plus an nC/nC+s
   augment row so PSUM holds a = nF + nC - 2*inter and dens = nF + nC + s.
 - x_dis via fp8(e4m3) DoubleRow matmuls (2 per subtile, 256-d contraction
   each) + a rank-3 bf16 augment (ones/xsq_hi/xsq_lo vs wsq/1/1) folding in
   ||x||^2 near-exactly and ||w||^2.
 - elementwise on DVE in bf16 (2x mode) over 2-granule pairs:
   t = xd*a, cden = t + dens, rc = 1/cden (ACT Reciprocal), qn = dens*rc,
   rs = rowsum (DVE reduce), rn = 1/rs, qf = qn*rn (per-subtile
   tensor_scalar, 4x mode). Pool runs ONLY local_scatter (GPSIMD library
   ops are exclusive), ACT does the PSUM->SBUF casts + reciprocal.
 - q|xd share one output tile per granule-pair -> one DMA per pair from SP;
   inputs ride the ACT HWDGE queue in 4-granule chunks to keep the HWDGE
   descriptor generator (shared, ~630ns/DMA) off the critical path.
"""

import os
import sys

import numpy as np

for _p in ("/opt/trn_rl_repo", os.path.expanduser("~/.axon_site/_ro/trn_rl_repo")):
    if os.path.isdir(_p) and _p not in sys.path:
        sys.path.insert(0, _p)

import concourse.bass as bass
import concourse.mybir as mybir
import concourse.tile as tile
from concourse import bacc, library_config
from concourse.bass_utils import run_bass_kernel_spmd

import ml_dtypes

NCORES = 8
B, D, K, LF, LC = 131072, 512, 64, 24, 64
V = 128            # ROI vocab == histogram bins
BS = B // NCORES   # fibers per core
SUB = 128          # fibers per subtile (partition dim)
GRAN = 512         # fibers per granule
NGRAN = BS // GRAN
NSUB = GRAN // SUB
NSLOT = BS // SUB  # 128 subtile slots per core
NPAIR = NGRAN // 2
CHUNK = 2          # granules per input-DMA chunk
SMOOTH = 1e-6

f32 = mybir.dt.float32
bf16 = mybir.dt.bfloat16
i16 = mybir.dt.int16
fp8 = mybir.dt.float8e4

bfdt = ml_dtypes.bfloat16
f8dt = ml_dtypes.float8_e4m3


def _build_nc(ws, d1=2, d2=2):
    """Per-core program. ws[t] = scatter index width (num_idxs, even) for
    subtile slot t; 0 = slot has no valid rois (skip scatter + histogram
    matmul). Shared across cores via the round-robin deal (host takes the
    max width over cores per slot)."""
    ws = tuple(int(w) for w in ws)
    offs = np.concatenate([[0], np.cumsum([2 * w for w in ws])])
    totw = int(offs[-1])
    # chunk boundaries in the scat tensor (CHUNK granules = 4*CHUNK slots)
    chunk_off = [int(offs[c * CHUNK * NSUB]) for c in range(NGRAN // CHUNK + 1)]

    nc = bacc.Bacc("TRN2", target_bir_lowering=False)

    xT8 = nc.dram_tensor("xT8", [D, BS], fp8, kind="ExternalInput")
    aug3 = nc.dram_tensor("aug3", [3, BS], bf16, kind="ExternalInput")
    scat = nc.dram_tensor("scat", [V, max(totw, 2)], i16, kind="ExternalInput")
    wT8 = nc.dram_tensor("wT8", [D, K], fp8, kind="ExternalInput")
    wsq3 = nc.dram_tensor("wsq3", [3, K], bf16, kind="ExternalInput")
    tbl2 = nc.dram_tensor("tbl2", [V, 2 * K], bf16, kind="ExternalInput")
    aug2 = nc.dram_tensor("aug2", [1, NSUB * 2 * K], bf16, kind="ExternalInput")

    # output: [p, pair, g2, (qf|xd), s, k] -> 2KB contiguous runs per pair
    out = nc.dram_tensor("out", [SUB, NPAIR, 2, 2, NSUB, K], bf16,
                         kind="ExternalOutput")

    xT_v = xT8[:].rearrange("(c p) n -> p c n", p=SUB)  # [128, 4, BS]

    with tile.TileContext(nc) as tc:
        with (
            tc.tile_pool(name="consts", bufs=1) as consts,
            tc.tile_pool(name="xin", bufs=3) as xin,
            tc.tile_pool(name="sin", bufs=3) as sin,
            tc.tile_pool(name="hist", bufs=10) as hist,
            tc.tile_pool(name="ew_ad", bufs=d1 + d2 + 2) as ew_ad,
            tc.tile_pool(name="ew_t", bufs=2) as ew_t,
            tc.tile_pool(name="ew_cd", bufs=d1 + 2) as ew_cd,
            tc.tile_pool(name="ew_rc", bufs=d1 + 2) as ew_rc,
            tc.tile_pool(name="ew_qn", bufs=d2 + 2) as ew_qn,
            tc.tile_pool(name="ew_rs", bufs=2) as ew_rs,
            tc.tile_pool(name="outs", bufs=d1 + d2 + 2) as outs,
            tc.tile_pool(name="psx", bufs=3, space="PSUM") as psx,
            tc.tile_pool(name="psi", bufs=3, space="PSUM") as psi,
        ):
            nc.gpsimd.load_library(library_config.local_scatter)

            def issue_x(ch):
                # first chunk split per-granule so granule 0 starts fast
                xt = xin.tile([SUB, 4, CHUNK * GRAN], fp8, tag="xt")
                if ch == 0:
                    for gi in range(CHUNK):
                        n0 = gi * GRAN
                        nc.scalar.dma_start(
                            out=xt[:, :, n0:n0 + GRAN],
                            in_=xT_v[:, :, n0:n0 + GRAN])
                else:
                    n0 = ch * CHUNK * GRAN
                    nc.scalar.dma_start(
                        out=xt, in_=xT_v[:, :, n0:n0 + CHUNK * GRAN])
                return xt

            def issue_scat(ch):
                # scat rides Pool's SWDGE: Pool self-feeds its scatter input
                so0, so1 = chunk_off[ch], chunk_off[ch + 1]
                st = sin.tile([V, max(so1 - so0, 2)], i16, tag="st")
                if so1 > so0:
                    nc.gpsimd.dma_start(out=st, in_=scat[:, so0:so1])
                return st

            xts = [issue_x(0), issue_x(1)]
            sts = [issue_scat(0), issue_scat(1)]

            c_wT8 = consts.tile([SUB, 4, K], fp8)
            nc.sync.dma_start(out=c_wT8,
                              in_=wT8[:].rearrange("(c p) k -> p c k", p=SUB))
            c_wsq3 = consts.tile([3, K], bf16)
            nc.sync.dma_start(out=c_wsq3, in_=wsq3[:])
            c_tbl2 = consts.tile([V, 2 * K], bf16)
            nc.sync.dma_start(out=c_tbl2, in_=tbl2[:])
            c_aug2 = consts.tile([1, NSUB * 2 * K], bf16)
            nc.sync.dma_start(out=c_aug2, in_=aug2[:])
            c_ones = consts.tile([1, SUB], bf16)
            nc.vector.memset(c_ones, 1.0)
            c_aug3 = consts.tile([3, BS], bf16)
            nc.sync.dma_start(out=c_aug3, in_=aug3[:])

            pend1 = []  # pairs awaiting t/cden/rc
            pend2 = []  # pairs awaiting qn/rs/rn/qf + out DMA

            def emit_stage1():
                pr, po1, ad1 = pend1.pop(0)
                xd_v = po1[:, :, 1, :, :]
                a_v = ad1[:, :, :, 0, :]
                d_v = ad1[:, :, :, 1, :]
                t_ = ew_t.tile([SUB, 2, NSUB, K], bf16, tag="t_")
                nc.vector.tensor_tensor(
                    out=t_, in0=xd_v, in1=a_v, op=mybir.AluOpType.mult)
                cden = ew_cd.tile([SUB, 2, NSUB, K], bf16, tag="cden")
                nc.vector.tensor_tensor(
                    out=cden, in0=t_, in1=d_v, op=mybir.AluOpType.add)
                rc = ew_rc.tile([SUB, 2, NSUB, K], bf16, tag="rc")
                with nc.allow_low_precision(reason="validated: q err 2.4e-3"):
                    nc.vector.reciprocal(out=rc, in_=cden)
                pend2.append((pr, po1, ad1, rc))

            def emit_stage2():
                pr, po2, ad2, rc2 = pend2.pop(0)
                d_v = ad2[:, :, :, 1, :]
                qn = ew_qn.tile([SUB, 2, NSUB, K], bf16, tag="qn")
                nc.vector.tensor_tensor(
                    out=qn, in0=d_v, in1=rc2, op=mybir.AluOpType.mult)
                rs = ew_rs.tile([SUB, 2, NSUB], f32, tag="rs")
                nc.vector.tensor_reduce(
                    out=rs, in_=qn,
                    axis=mybir.AxisListType.X, op=mybir.AluOpType.add)
                rn = ew_rs.tile([SUB, 2, NSUB], f32, tag="rn")
                nc.vector.reciprocal(out=rn, in_=rs)
                for i in range(2):
                    for s in range(NSUB):
                        nc.vector.tensor_scalar(
                            out=po2[:, i, 0, s, :], in0=qn[:, i, s, :],
                            scalar1=rn[:, i, s:s + 1], scalar2=None,
                            op0=mybir.AluOpType.mult)
                nc.sync.dma_start(out=out[:, pr], in_=po2[:])

            po = None
            for g in range(NGRAN):
                ch, gin = divmod(g, CHUNK)
                if gin == 0 and ch + 2 < NGRAN // CHUNK:
                    xts.append(issue_x(ch + 2))
                    sts.append(issue_scat(ch + 2))
                xt, st = xts[ch], sts[ch]

                if g % 2 == 0:
                    po = outs.tile([SUB, 2, 2, NSUB, K], bf16, tag="po")
                i = g % 2

                psum_x = psx.tile([SUB, NSUB, K], f32, tag="px")
                psum_ad = psi.tile([SUB, NSUB, 2, K], f32, tag="pad")

                # x_dis matmuls first: PE work with no scatter dependency
                for s in range(NSUB):
                    f0 = gin * GRAN + s * SUB
                    for c in range(2):
                        nc.tensor.matmul(
                            psum_x[:, s, :],
                            lhsT=xt[:, 2 * c:2 * c + 2, f0:f0 + SUB],
                            rhs=c_wT8[:, 2 * c:2 * c + 2, :],
                            start=(c == 0), stop=False,
                            perf_mode=mybir.MatmulPerfMode.DoubleRow,
                        )
                    b0 = g * GRAN + s * SUB
                    nc.tensor.matmul(
                        psum_x[:, s, :],
                        lhsT=c_aug3[:, b0:b0 + SUB], rhs=c_wsq3,
                        start=False, stop=True,
                    )

                hts = [None] * NSUB
                for s in range(NSUB):
                    t = g * NSUB + s
                    w = ws[t]
                    if w == 0:
                        continue
                    o = int(offs[t]) - chunk_off[ch]
                    ht = hist.tile([V, SUB], bf16, tag="ht")
                    nc.gpsimd.local_scatter(
                        out_ap=ht[:],
                        data_ap=st[:, o + w:o + 2 * w].bitcast(bf16),
                        idxs_ap=st[:, o:o + w],
                        channels=V, num_elems=SUB, num_idxs=w,
                    )
                    hts[s] = ht

                # one granule-wide aug matmul seeds a/dens for all subtiles;
                # per-subtile histogram matmuls accumulate on top
                live = [s for s in range(NSUB) if hts[s] is not None]
                nc.tensor.matmul(
                    psum_ad[:], lhsT=c_ones, rhs=c_aug2,
                    start=True, stop=(not live), skip_group_check=True,
                )
                for n, s in enumerate(live):
                    nc.tensor.matmul(
                        psum_ad[:, s], lhsT=hts[s][:], rhs=c_tbl2,
                        start=False, stop=(n == len(live) - 1),
                        skip_group_check=True,
                    )

                # PSUM -> SBUF casts on ACT
                nc.scalar.copy(out=po[:, i, 1], in_=psum_x)
                if i == 0:
                    ad = ew_ad.tile([SUB, 2, NSUB, 2, K], bf16, tag="ad")
                nc.scalar.copy(out=ad[:, i], in_=psum_ad)

                if i == 1:
                    pend1.append((g // 2, po, ad))
                    if len(pend1) > d1:
                        emit_stage1()
                    if len(pend2) > d2:
                        emit_stage2()

            while pend1 or pend2:
                if pend1:
                    emit_stage1()
                if pend2:
                    emit_stage2()

    nc.finalize()
    return nc


_NC_CACHE = None
_NC_KEY = None
_LAST = None


def _get_nc(ws=None, **opts):
    global _NC_CACHE, _NC_KEY
    if ws is None:
        assert _NC_CACHE is not None
        return _NC_CACHE
    key = (tuple(int(w) for w in ws), tuple(sorted(opts.items())))
    if _NC_CACHE is None or _NC_KEY != key:
        _NC_CACHE = _build_nc(tuple(int(w) for w in ws), **opts)
        _NC_KEY = key
    return _NC_CACHE


def _scatter_tables(fiber_rois, fiber_lens, deal):
    """Per-core scatter tables. Returns (ws, scats) where ws[t] is the even
    index width for slot t (max over cores) and scats[c] is the packed
    [V, totw] int16 array (idx block | bf16-bits data block per slot)."""
    percore = []  # percore[c][t] = (bins, fibs, counts)
    ws = np.zeros(NSLOT, np.int64)
    ar = np.arange(LF)
    for c in range(NCORES):
        slots = []
        for t in range(NSLOT):
            rows = deal[t, c]
            lens = fiber_lens[rows]
            rois = fiber_rois[rows]
            mask = ar[None, :] < lens[:, None]
            fib = np.repeat(np.arange(SUB), LF).reshape(SUB, LF)[mask]
            vals = rois[mask]
            if vals.size == 0:
                slots.append(None)
                continue
            key = fib.astype(np.int64) * V + vals
            uk, cnt = np.unique(key, return_counts=True)
            bins = (uk % V).astype(np.int64)
            fibs = (uk // V).astype(np.int64)
            order = np.argsort(bins, kind="stable")
            bins, fibs, cnt = bins[order], fibs[order], cnt[order]
            bc = np.bincount(bins, minlength=V)
            ws[t] = max(ws[t], bc.max())
            slots.append((bins, fibs, cnt))
        percore.append(slots)
    ws = ((ws + 1) // 2 * 2).astype(np.int64)  # num_idxs must be even
    offs = np.concatenate([[0], np.cumsum(2 * ws)])
    totw = max(int(offs[-1]), 2)
    scats = []
    for c in range(NCORES):
        sc = np.zeros((V, totw), np.int16)
        sc[:, :] = -1  # idx padding; harmless in data blocks (overwritten)
        for t in range(NSLOT):
            w = int(ws[t])
            if w == 0:
                continue
            o = int(offs[t])
            idx = np.full((V, w), -1, np.int16)
            dat = np.zeros((V, w), bfdt)
            if percore[c][t] is not None:
                bins, fibs, cnt = percore[c][t]
                col = np.zeros(V, np.int64)
                pos = np.empty(len(bins), np.int64)
                for n, v in enumerate(bins):
                    pos[n] = col[v]
                    col[v] += 1
                idx[bins, pos] = fibs.astype(np.int16)
                dat[bins, pos] = cnt.astype(np.float32)
            sc[:, o:o + w] = idx
            sc[:, o + w:o + 2 * w] = dat.view(np.int16)
        scats.append(sc)
    return ws, scats


def kernel(x, weight, fiber_rois, fiber_lens, cluster_rois, cluster_lens):
    x = np.asarray(x, np.float32)
    weight = np.asarray(weight, np.float32)
    fiber_rois = np.asarray(fiber_rois, np.int32)
    fiber_lens = np.asarray(fiber_lens, np.int32)
    cluster_rois = np.asarray(cluster_rois, np.int32)
    cluster_lens = np.asarray(cluster_lens, np.int32)

    # K-side host prep (tiny): cluster histogram table, norms, constants
    mC = (np.arange(LC)[None, :] < cluster_lens[:, None])
    histC = np.zeros((K, V), np.float32)
    for k in range(K):
        histC[k] = np.bincount(cluster_rois[k][mC[k]], minlength=V)
    nC = cluster_lens.astype(np.float32)
    tbl2 = np.concatenate(
        [1.0 - 2.0 * histC.T, np.ones((V, K), np.float32)], axis=1
    ).astype(bfdt)
    aug2 = np.tile(np.concatenate([nC, nC + SMOOTH]), NSUB)[None, :].astype(bfdt)
    wsq = (weight * weight).sum(1).astype(np.float32)
    wsq3 = np.stack([wsq, np.ones(K, np.float32), np.ones(K, np.float32)])
    wsq3 = wsq3.astype(bfdt)
    wT8 = np.ascontiguousarray((-2.0 * weight.T)).astype(f8dt)  # [D, K]

    # fiber-side layout: sort by length, deal round-robin across cores so
    # every core shares one compile-time profile
    order = np.argsort(fiber_lens, kind="stable")
    deal = order.reshape(NSLOT, NCORES, SUB)  # [slot, core, row]

    ws, scats = _scatter_tables(fiber_rois, fiber_lens, deal)

    xsq = np.einsum("bd,bd->b", x, x).astype(np.float32)
    xsq_hi = xsq.astype(bfdt)
    xsq_lo = (xsq - xsq_hi.astype(np.float32)).astype(bfdt)
    ones_b = np.ones(B, bfdt)
    x_f8 = x.astype(f8dt)

    nc = _get_nc(ws)
    in_maps = []
    perms = []
    for ci in range(NCORES):
        perm = deal[:, ci, :].reshape(BS)
        perms.append(perm)
        in_maps.append({
            "xT8": np.ascontiguousarray(x_f8[perm].T),
            "aug3": np.ascontiguousarray(
                np.stack([ones_b[perm], xsq_hi[perm], xsq_lo[perm]])),
            "scat": scats[ci],
            "wT8": wT8,
            "wsq3": wsq3,
            "tbl2": tbl2,
            "aug2": aug2,
        })

    res = run_bass_kernel_spmd(nc, in_maps, core_ids=list(range(NCORES)))
    global _LAST
    _LAST = res
    q = np.empty((B, K), np.float32)
    xd = np.empty((B, K), np.float32)
    for ci in range(NCORES):
        # out[p, pair, g2, c, s, k]; fiber of slot t = (pair*2+g2)*NSUB+s,
        # partition p is perm[t*SUB + p]
        o = res.results[ci]["out"].astype(np.float32)
        o = o.reshape(SUB, NSLOT // NSUB, 2, NSUB, K)  # [p, g, c, s, k]
        qo = o[:, :, 0].transpose(1, 2, 0, 3).reshape(BS, K)
        xo = o[:, :, 1].transpose(1, 2, 0, 3).reshape(BS, K)
        q[perms[ci]] = qo
        xd[perms[ci]] = xo
    return (q, xd)
